# revision 27
# baseline (speedup 1.0000x reference)
"""APPNP (K-step PPR propagation) GNN on 8 Trainium2 NeuronCores.

Strategy (1D node-parallel, per the sharding hint):
  - dst nodes sharded across 8 cores; per core TILES tiles of 128 dst rows.
  - per propagation step each core bulk-gathers source-node feature rows for
    its in-edges with InstDMAGatherAnt (int16 indices -> BLOCKS sub-tables of
    <=32768 rows), reduces the edge messages into per-dst aggregates with
    one-hot selector matmuls on the TensorEngine (selectors built on-device
    via is_equal against an iota tile), applies the APPNP epilogue on the
    VectorEngine, then AllGathers the updated norm-scaled feature shards into
    every core's gather table.

One SPMD Bass graph for all 8 cores; per-core edge index/offset streams are
passed via in_maps.  All edge bookkeeping is host-side numpy.
"""

from dataclasses import dataclass
from contextlib import ExitStack

import numpy as np

import concourse.bass as bass
import concourse.bacc as bacc
import concourse.mybir as mybir
from concourse.ap import AP


# ----------------------------------------------------------------------------
# configuration
# ----------------------------------------------------------------------------
@dataclass(frozen=True)
class Cfg:
    C: int = 8            # cores
    D: int = 64           # feature dim
    PER_RAW: int = 12500  # raw nodes per core
    TILES: int = 108      # dst tiles (of 128) per core
    BLOCKS: int = 4       # src sub-tables (rows must fit int16)
    CAP: int = 512        # slots per (tile, block) group (multiple of 128)
    G: int = 6144         # gather window slots (multiple of 128)
    BPW: int = 7          # chunks per selector-build instruction
    K: int = 10           # propagation steps
    ALPHA: float = 0.1
    # ablation flags (timing experiments only; results become wrong)
    ab_no_coll: bool = False
    ab_no_gather: bool = False
    ab_no_pe: bool = False
    ab_no_build: bool = False

    @property
    def PERP(self):
        return self.TILES * 128

    @property
    def NODES_P(self):
        return self.C * self.PERP

    @property
    def BROWS(self):
        return self.NODES_P // self.BLOCKS

    @property
    def CPT(self):
        return self.CAP // 128

    @property
    def SLOTS_B(self):
        return self.TILES * self.CAP

    @property
    def WIN_B(self):
        assert self.SLOTS_B % self.G == 0
        return self.SLOTS_B // self.G

    @property
    def WSLOT(self):
        return self.G // 128

    @property
    def CW(self):  # matmul chunks per window
        return self.G // 128

    @property
    def NWIN(self):
        return self.BLOCKS * self.WIN_B


FULL = Cfg()
OOB = 500.0  # dstoff sentinel for pad slots (outside iota range 0..127)

F32 = mybir.dt.float32
I16 = mybir.dt.int16


# ----------------------------------------------------------------------------
# host-side preprocessing
# ----------------------------------------------------------------------------
def preprocess(x, W, b, src, dst, cfg: Cfg):
    N, D = x.shape
    assert N == cfg.C * cfg.PER_RAW and D == cfg.D
    src = np.asarray(src, np.int64)
    dst = np.asarray(dst, np.int64)
    x = np.asarray(x, np.float32)
    b = np.asarray(b, np.float32)

    deg_out = np.bincount(src, minlength=N).astype(np.float32)
    deg_in = np.bincount(dst, minlength=N).astype(np.float32)
    ns = np.where(deg_out > 0, 1.0 / np.sqrt(np.maximum(deg_out, 1.0)), 0.0)
    nd = np.where(deg_in > 0, 1.0 / np.sqrt(np.maximum(deg_in, 1.0)), 0.0)
    ns = ns.astype(np.float32)
    nd = nd.astype(np.float32)

    # Balanced dst->tile assignment per core: keep every (tile, block) edge
    # group under CAP so the static SPMD chunk schedule stays tight.
    # node (c, loc) -> (t, p); table row = c*PERP + p*TILES + t.
    dst_c = dst // cfg.PER_RAW
    dst_loc = dst % cfg.PER_RAW
    src_c = src // cfg.PER_RAW
    src_loc = src % cfg.PER_RAW

    # Gather block q == tile-quarter q so each quarter's AllGather can
    # pipeline against the next step's block-q gathers.  A node's quarter is
    # fixed up front (loc // NPQ), so per-dst block-degree vectors are known
    # before tile assignment and balancing stays exact.
    NQ = cfg.BLOCKS
    TPQ = cfg.TILES // NQ
    NPQ = cfg.PER_RAW // NQ
    assert cfg.TILES % NQ == 0 and cfg.PER_RAW % NQ == 0

    perm_t = np.empty((cfg.C, cfg.PER_RAW), np.int64)
    perm_p = np.empty((cfg.C, cfg.PER_RAW), np.int64)
    src_q = src_loc // NPQ  # fixed quarter of each edge's source node
    for c in range(cfg.C):
        sel = np.nonzero(dst_c == c)[0]
        locs = dst_loc[sel]
        degv = np.zeros((cfg.PER_RAW, cfg.BLOCKS), np.int64)
        np.add.at(degv, (locs, src_q[sel]), 1)
        for q in range(NQ):
            lo_n, hi_n = q * NPQ, (q + 1) * NPQ
            order_d = lo_n + np.argsort(-degv[lo_n:hi_n].sum(1), kind="stable")
            loads = np.zeros((TPQ, cfg.BLOCKS), np.int64)
            cnt = np.zeros(TPQ, np.int64)
            for d in order_d:
                v = degv[d]
                scores = (loads + v).max(1)
                scores[cnt >= 128] = 1 << 30
                t = int(np.argmin(scores))
                perm_t[c, d] = q * TPQ + t
                perm_p[c, d] = cnt[t]
                loads[t] += v
                cnt[t] += 1
            if (loads > cfg.CAP).any():
                raise RuntimeError("balance failed")

    # table row (q-major): q*BROWS + c*(128*TPQ) + p*TPQ + (t % TPQ)
    s_t = perm_t[src_c, src_loc]
    s_p = perm_p[src_c, src_loc]
    blk_e = s_t // TPQ
    idx_e = src_c * (128 * TPQ) + s_p * TPQ + (s_t % TPQ)
    assert idx_e.max() < 32768

    dst_t = perm_t[dst_c, dst_loc]
    dst_p = perm_p[dst_c, dst_loc]

    iota = np.tile(np.arange(128, dtype=np.float32), (128, 1))
    brep01 = np.tile(cfg.ALPHA * b[None, :], (128, 1))
    w_in = np.ascontiguousarray(np.asarray(W, np.float32))

    in_maps = []
    for c in range(cfg.C):
        m = dst_c == c
        e_idx = idx_e[m].astype(np.int16)
        e_blk = blk_e[m]
        e_t = dst_t[m]
        e_p = dst_p[m].astype(np.float32)

        key = e_blk * cfg.TILES + e_t
        order = np.argsort(key, kind="stable")
        key_s = key[order]
        counts = np.bincount(key_s, minlength=cfg.BLOCKS * cfg.TILES)
        if counts.max() > cfg.CAP:
            raise RuntimeError(f"group overflow: {counts.max()} > {cfg.CAP}")
        starts = np.concatenate([[0], np.cumsum(counts)[:-1]])
        within = np.arange(len(key_s)) - starts[key_s]
        slot = key_s * cfg.CAP + within

        idx_stream = np.zeros(cfg.BLOCKS * cfg.SLOTS_B, np.int16)
        doff_stream = np.full(cfg.BLOCKS * cfg.SLOTS_B, OOB, np.float32)
        idx_stream[slot] = e_idx[order]
        doff_stream[slot] = e_p[order]

        idx_w = idx_stream.reshape(cfg.NWIN, cfg.G)
        idx16 = idx_w.reshape(cfg.NWIN, cfg.G // 16, 16).transpose(0, 2, 1)
        # Q7 wants the 16-partition index block replicated across all 8
        # gpsimd cores (128 partitions total)
        idx_in = np.ascontiguousarray(np.tile(idx16, (1, 8, 1)))
        doff_w = doff_stream.reshape(cfg.NWIN, cfg.G)
        doff_in = np.ascontiguousarray(
            doff_w.reshape(cfg.NWIN, cfg.WSLOT, 128).transpose(0, 2, 1)
        )

        lo, hi = c * cfg.PER_RAW, (c + 1) * cfg.PER_RAW
        col = perm_t[c] * 128 + perm_p[c]  # node loc -> xT column (t*128+p)
        xT = np.zeros((cfg.D, cfg.PERP), np.float32)
        xT[:, col] = x[lo:hi].T
        ns_in = np.zeros((128, cfg.TILES), np.float32)
        nd9_in = np.zeros((128, cfg.TILES), np.float32)
        ns_in[perm_p[c], perm_t[c]] = ns[lo:hi]
        nd9_in[perm_p[c], perm_t[c]] = (1.0 - cfg.ALPHA) * nd[lo:hi]
        # bns[p, t, c] = b[c] * ns[node(t,p)]  (for fused init epilogue)
        bns = np.ascontiguousarray(
            ns_in[:, :, None] * b[None, None, :]
        )  # [128, TILES, D]

        in_maps.append(
            {
                "xT": xT,
                "w": w_in,
                "brep01": brep01,
                "bns": bns,
                "iota": iota,
                "ns": ns_in,
                "nd9": nd9_in,
                "idx": idx_in,
                "doff": doff_in,
            }
        )
    return in_maps, (perm_t, perm_p)


def assemble_output(outs, cfg: Cfg, perms):
    perm_t, perm_p = perms
    N = cfg.C * cfg.PER_RAW
    full = np.empty((N, cfg.D), np.float32)
    for c in range(cfg.C):
        o = np.asarray(outs[c]).reshape(128, cfg.TILES, cfg.D)
        full[c * cfg.PER_RAW : (c + 1) * cfg.PER_RAW] = o[perm_p[c], perm_t[c]]
    return full


# ----------------------------------------------------------------------------
# graph builder
# ----------------------------------------------------------------------------
class Builder:
    def __init__(self, cfg: Cfg):
        self.cfg = cfg
        self.nc = bacc.Bacc("TRN2", target_bir_lowering=False)
        self.prog = {"sp": [], "gps": [], "pe": [], "dve": []}
        self.sp_n = 0
        self.pe_n = 0
        self.dve_n = 0
        self.cc_n = 0
        self.guse = [0, 0]
        self.uses_idx = [0, 0]
        self.uses_doff = [0, 0]
        self.st_n = 0
        self._last_wait = {}

    def op(self, engine, fn):
        self.prog[engine].append(fn)

    def wait(self, engine, s, v):
        """Emit wait_ge, deduping against earlier >= waits on this engine."""
        if v <= 0:
            return
        key = (engine, id(s))
        if self._last_wait.get(key, -1) >= v:
            return
        self._last_wait[key] = v
        self.prog[engine].append(lambda eng, s=s, v=v: eng.wait_ge(s, v))

    def build(self):
        cfg, nc = self.cfg, self.nc
        D = cfg.D

        xT_d = nc.declare_dram_parameter("xT", [D, cfg.PERP], F32, isOutput=False)
        w_d = nc.declare_dram_parameter("w", [D, D], F32, isOutput=False)
        brep01_d = nc.declare_dram_parameter("brep01", [128, D], F32, isOutput=False)
        bns_d = nc.declare_dram_parameter(
            "bns", [128, cfg.TILES, D], F32, isOutput=False
        )
        iota_d = nc.declare_dram_parameter("iota", [128, 128], F32, isOutput=False)
        ns_d = nc.declare_dram_parameter("ns", [128, cfg.TILES], F32, isOutput=False)
        nd9_d = nc.declare_dram_parameter("nd9", [128, cfg.TILES], F32, isOutput=False)
        idx_d = nc.declare_dram_parameter(
            "idx", [cfg.NWIN, 128, cfg.G // 16], I16, isOutput=False
        )
        doff_d = nc.declare_dram_parameter(
            "doff", [cfg.NWIN, 128, cfg.WSLOT], F32, isOutput=False
        )
        out_d = nc.declare_dram_parameter("out", [128, cfg.TILES, D], F32, isOutput=True)

        TPQ0 = cfg.TILES // cfg.BLOCKS
        shard_q = [
            nc.dram_tensor(f"shard{q}", [128, TPQ0, D], F32)
            for q in range(cfg.BLOCKS)
        ]
        table_d = nc.dram_tensor("table", [cfg.NODES_P, D], F32, addr_space="Shared")

        ctx = ExitStack()
        sb = lambda name, shape, dt=F32: ctx.enter_context(
            nc.sbuf_tensor(name, shape, dt)
        )
        xt_sb = sb("xt", [64, cfg.PERP])
        w_sb = sb("wsb", [64, D])
        brep01_sb = sb("brep01sb", [128, D])
        bns_sb = sb("bnssb", [128, cfg.TILES * D])
        iota_sb = sb("iotasb", [128, 128])
        ns_sb = sb("nssb", [128, cfg.TILES])
        nd9_sb = sb("nd9sb", [128, cfg.TILES])
        h0a_sb = sb("h0a", [128, cfg.TILES * D])
        part_sb = sb("part", [128, cfg.TILES * D])
        stage_sb = sb("stage", [128, cfg.TILES * D])
        NPS = 8
        NS8 = 4
        msg_sb = [sb(f"msg{i}", [128, cfg.WSLOT, D]) for i in range(2)]
        s8_sb = [sb(f"s8_{i}", [128, cfg.BPW * 128]) for i in range(NS8)]
        doff_sb = [sb(f"doffsb{i}", [128, cfg.WSLOT]) for i in range(2)]
        idx_sb = [sb(f"idxsb{i}", [128, cfg.G // 16], I16) for i in range(2)]
        psum = [
            ctx.enter_context(nc.psum_tensor(f"ps{i}", [128, 512], F32))
            for i in range(NPS)
        ]

        sem_sp = ctx.enter_context(nc.semaphore("s_sp"))
        sem_g = [
            ctx.enter_context(nc.semaphore("s_g0")),
            ctx.enter_context(nc.semaphore("s_g1")),
        ]
        sem_idx = [
            ctx.enter_context(nc.semaphore("s_ix0")),
            ctx.enter_context(nc.semaphore("s_ix1")),
        ]
        sem_doff = [
            ctx.enter_context(nc.semaphore("s_do0")),
            ctx.enter_context(nc.semaphore("s_do1")),
        ]
        sem_st = [
            ctx.enter_context(nc.semaphore(f"s_st{i}")) for i in range(cfg.BLOCKS)
        ]
        sem_cc = ctx.enter_context(nc.semaphore("s_cc"))
        sem_pe = ctx.enter_context(nc.semaphore("s_pe"))
        sem_dve = ctx.enter_context(nc.semaphore("s_dve"))

        # gpsimd: load the Q7 library that provides DMAGatherAnt; pin the
        # num_idxs register once (to_reg per gather would exhaust the pool)
        holder = {}

        def f(eng):
            from concourse import library_config

            nc.gpsimd.load_library(library_config.mlp)
            holder["greg"] = nc.gpsimd.to_reg(cfg.G)

        self.op("gps", f)

        # ---------------- init: constant DMAs ----------------
        def sp_dma(mk):
            def f(eng, mk=mk):
                out_ap, in_ap = mk()
                nc.sync.dma_start(out=out_ap, in_=in_ap).then_inc(sem_sp, 16)

            self.op("sp", f)
            self.sp_n += 16
            return self.sp_n

        sp_dma(lambda: (xt_sb[:, :], xT_d[:, :]))
        sp_dma(lambda: (w_sb[:, :], w_d[:, :]))
        sp_dma(lambda: (brep01_sb[:, :], brep01_d[:, :]))
        sp_dma(
            lambda: (
                bns_sb[:, :].rearrange("p (t c) -> p t c", c=D),
                bns_d[:, :, :],
            )
        )
        sp_dma(lambda: (iota_sb[:, :], iota_d[:, :]))
        sp_dma(lambda: (ns_sb[:, :], ns_d[:, :]))
        init_dma_done = sp_dma(lambda: (nd9_sb[:, :], nd9_d[:, :]))

        # ---------------- init: h0 = x @ W + b; seed stage/h0a ----------------
        init_tile_done = {}
        self.wait("pe", sem_sp, init_dma_done)
        for t in range(cfg.TILES):
            slot = t % NPS
            if t >= NPS:
                self.wait("pe", sem_dve, init_tile_done[t - NPS])

            def f(eng, t=t, slot=slot):
                nc.tensor.matmul(
                    out=psum[slot][:, :D],
                    lhsT=xt_sb[:, t * 128 : (t + 1) * 128],
                    rhs=w_sb[:, :],
                    start=True,
                    stop=True,
                ).then_inc(sem_pe, 1)

            self.op("pe", f)
            self.pe_n += 1

            self.wait("dve", sem_pe, self.pe_n)
            self.wait("dve", sem_sp, init_dma_done)

            def f(eng, t=t, slot=slot):
                # h0a_t = alpha*(xW) + alpha*b ; stage_t = ns*(xW) + ns*b
                nc.vector.scalar_tensor_tensor(
                    out=h0a_sb[:, t * D : (t + 1) * D],
                    in0=psum[slot][:, :D],
                    scalar=cfg.ALPHA,
                    in1=brep01_sb[:, :],
                    op0=mybir.AluOpType.mult,
                    op1=mybir.AluOpType.add,
                ).then_inc(sem_dve, 1)
                nc.vector.scalar_tensor_tensor(
                    out=stage_sb[:, t * D : (t + 1) * D],
                    in0=psum[slot][:, :D],
                    scalar=ns_sb[:, t : t + 1],
                    in1=bns_sb[:, t * D : (t + 1) * D],
                    op0=mybir.AluOpType.mult,
                    op1=mybir.AluOpType.add,
                ).then_inc(sem_dve, 1)

            self.op("dve", f)
            self.dve_n += 2
            init_tile_done[t] = self.dve_n
        init_done_dve = self.dve_n

        TPQ = cfg.TILES // cfg.BLOCKS
        self.st_q = [0] * cfg.BLOCKS

        def stage_quarter(dram3, q):
            # dram3 None -> this quarter's shard tensor; else out_d slice
            a, b2 = q * TPQ, (q + 1) * TPQ

            def f(eng, dram3=dram3, q=q, a=a, b2=b2):
                out_ap = (
                    shard_q[q][:, :, :] if dram3 is None else dram3[:, a:b2, :]
                )
                nc.sync.dma_start(
                    out=out_ap,
                    in_=stage_sb[:, a * D : b2 * D].rearrange(
                        "p (t c) -> p t c", c=D
                    ),
                ).then_inc(sem_st[q], 16)

            self.op("sp", f)
            self.st_q[q] += 16
            return self.st_q[q]

        for q in range(cfg.BLOCKS):
            self.wait("sp", sem_dve, init_tile_done[(q + 1) * TPQ - 1])
            stage_quarter(None, q)

        # ---------------- propagation steps ----------------
        win_last_chunk_pe = {}
        build_done = {}
        build_last_pe = {}
        build_of_chunk = {}
        last_doff_builds = {}
        spill_done = {}
        bi_global = 0
        q_global = 0

        blk_guse = {}

        def emit_quarter_tail(k, qq):
            # epilogue waves + stage dma for tile quarter qq of step k;
            # runs while the B3 pass continues on later quarters.
            a = qq * TPQ
            self.wait("dve", sem_dve, self.dve_n)     # B3 adds retired
            self.wait("dve", sem_st[qq], 16 * k)      # stage quarter free
            for t in range(a, a + TPQ):

                def f(eng, t=t):
                    nc.vector.scalar_tensor_tensor(
                        out=stage_sb[:, t * D : (t + 1) * D],
                        in0=part_sb[:, t * D : (t + 1) * D],
                        scalar=nd9_sb[:, t : t + 1],
                        in1=h0a_sb[:, t * D : (t + 1) * D],
                        op0=mybir.AluOpType.mult,
                        op1=mybir.AluOpType.add,
                    ).then_inc(sem_dve, 1)

                self.op("dve", f)
                self.dve_n += 1
            if k < cfg.K:
                self.wait("dve", sem_dve, self.dve_n)  # wave barrier
                for t in range(a, a + TPQ):

                    def f(eng, t=t):
                        nc.vector.tensor_scalar_mul(
                            stage_sb[:, t * D : (t + 1) * D],
                            stage_sb[:, t * D : (t + 1) * D],
                            ns_sb[:, t : t + 1],
                        ).then_inc(sem_dve, 1)

                    self.op("dve", f)
                    self.dve_n += 1
            # SP: stage quarter (shard for next step, out on last step)
            self.wait("sp", sem_dve, self.dve_n)
            if not cfg.ab_no_coll:
                self.wait("sp", sem_cc, 4 * (k - 1) + qq + 1)  # shard-q WAR
            stage_quarter(None if k < cfg.K else out_d, qq)

        for k in range(1, cfg.K + 1):
            for B in range(cfg.BLOCKS):
                # quarter collective B: shard quarter -> table block rows.
                # Pipelines: issued just before this block's gathers; waits
                # only on quarter-B stage of step k-1 and on step k-1's
                # block-B gathers having drained (table WAR).
                self.wait("gps", sem_st[B], 16 * k)
                snap = blk_guse.get((k - 1, B))
                if snap is not None:
                    self.wait("gps", sem_g[0], 16 * snap[0])
                    self.wait("gps", sem_g[1], 16 * snap[1])

                def f(eng, B=B):
                    if cfg.ab_no_coll:
                        return
                    nc.gpsimd.collective_compute(
                        "AllGather",
                        mybir.AluOpType.bypass,
                        replica_groups=[list(range(cfg.C))],
                        ins=[shard_q[B].ap().opt()],
                        outs=[
                            table_d[
                                B * cfg.BROWS : (B + 1) * cfg.BROWS, :
                            ].opt()
                        ],
                    ).then_inc(sem_cc, 1)

                self.op("gps", f)
                if not cfg.ab_no_coll:
                    self.cc_n += 1
                cc_now = self.cc_n

                pass_wait_done = False
                for w in range(cfg.WIN_B):
                    g = B * cfg.WIN_B + w
                    par = g % 2
                    prev = (k, g - 2) if g - 2 >= 0 else (k - 1, g - 2 + cfg.NWIN)

                    # SP: idx window dma (WAR: last gather on this parity done)
                    self.wait("sp", sem_g[par], 16 * self.guse[par])

                    def f(eng, g=g, par=par):
                        nc.sync.dma_start(
                            out=idx_sb[par][:, :], in_=idx_d[g, :, :]
                        ).then_inc(sem_idx[par], 16)

                    self.op("sp", f)
                    self.uses_idx[par] += 16
                    idx_dma_done = self.uses_idx[par]

                    # SP: doff window dma (WAR: builds of window g-2 done)
                    if prev in last_doff_builds:
                        self.wait("sp", sem_dve, build_done[last_doff_builds[prev]])

                    def f(eng, g=g, par=par):
                        nc.sync.dma_start(
                            out=doff_sb[par][:, :], in_=doff_d[g, :, :]
                        ).then_inc(sem_doff[par], 16)

                    self.op("sp", f)
                    self.uses_doff[par] += 16
                    doff_dma_done = self.uses_doff[par]

                    # GPS: gather
                    self.wait("gps", sem_cc, cc_now)
                    self.wait("gps", sem_idx[par], idx_dma_done)
                    if prev in win_last_chunk_pe:
                        self.wait("gps", sem_pe, win_last_chunk_pe[prev])

                    def f(eng, B=B, par=par):
                        if cfg.ab_no_gather:
                            nc.gpsimd.memset(msg_sb[par][0:1, 0:1, 0:1], 0).then_inc(
                                sem_g[par], 16
                            )
                            return
                        nc.gpsimd.dma_gather(
                            out_ap=msg_sb[par][:, :, :],
                            in_ap=table_d[B * cfg.BROWS : (B + 1) * cfg.BROWS, :],
                            idxs_ap=idx_sb[par][:, :],
                            num_idxs=cfg.G,
                            num_idxs_reg=holder["greg"],
                            elem_size=D,
                            single_packet=False,
                        ).then_inc(sem_g[par], 16)

                    self.op("gps", f)
                    self.guse[par] += 1
                    g_thresh = 16 * self.guse[par]

                    # DVE builds + PE chunks + DVE spills, interleaved
                    first_c = w * cfg.CW
                    for cw in range(cfg.CW):
                        if cw % cfg.BPW == 0:
                            bw = cw // cfg.BPW
                            nch = min(cfg.BPW, cfg.CW - cw)
                            sbi = bi_global % NS8
                            self.wait("dve", sem_doff[par], doff_dma_done)
                            if (bi_global - NS8) in build_last_pe:
                                self.wait(
                                    "dve", sem_pe, build_last_pe[bi_global - NS8]
                                )

                            def f(eng, par=par, bw=bw, nch=nch, sbi=sbi):
                                if cfg.ab_no_build:
                                    nc.vector.memset(
                                        s8_sb[sbi][0:1, 0:1], 0
                                    ).then_inc(sem_dve, 1)
                                    return
                                s8 = s8_sb[sbi][:, :]
                                out_ap = AP(
                                    tensor=s8.tensor,
                                    offset=s8.offset,
                                    ap=[list(s8.ap[0]), [128, nch], [1, 128]],
                                )
                                in0 = doff_sb[par][
                                    :, bw * cfg.BPW : bw * cfg.BPW + nch
                                ].to_broadcast([128, nch, 128])
                                io = iota_sb[:, :]
                                in1 = AP(
                                    tensor=io.tensor,
                                    offset=io.offset,
                                    ap=[list(io.ap[0]), [0, nch], [1, 128]],
                                )
                                nc.vector.tensor_tensor(
                                    out=out_ap,
                                    in0=in0,
                                    in1=in1,
                                    op=mybir.AluOpType.is_equal,
                                ).then_inc(sem_dve, 1)

                            self.op("dve", f)
                            self.dve_n += 1
                            build_done[bi_global] = self.dve_n
                            for cc2 in range(cw, cw + nch):
                                build_of_chunk[(k, g, cc2)] = bi_global
                            last_doff_builds[(k, g)] = bi_global
                            bi_global += 1

                        c_glob = first_c + cw
                        t = c_glob // cfg.CPT
                        j = c_glob % cfg.CPT
                        bi = build_of_chunk[(k, g, cw)]
                        sbi = bi % NS8
                        self.wait("pe", sem_g[par], g_thresh)
                        self.wait("pe", sem_dve, build_done[bi])
                        if j == 0:
                            qh = q_global - NPS
                            if qh >= 0:
                                self.wait("pe", sem_dve, spill_done[qh])
                            else:
                                self.wait("pe", sem_dve, init_done_dve)
                            q_global += 1
                        slot = (q_global - 1) % NPS

                        def f(eng, par=par, cw=cw, j=j, sbi=sbi, slot=slot):
                            if cfg.ab_no_pe:
                                nc.tensor.memset(psum[slot][0:1, 0:1], 0).then_inc(
                                    sem_pe, 1
                                )
                                return
                            col = (cw % cfg.BPW) * 128
                            nc.tensor.matmul(
                                out=psum[slot][:, :D],
                                lhsT=s8_sb[sbi][:, col : col + 128],
                                rhs=msg_sb[par][:, cw, :],
                                start=(j == 0),
                                stop=(j == cfg.CPT - 1),
                            ).then_inc(sem_pe, 1)

                        self.op("pe", f)
                        self.pe_n += 1
                        build_last_pe[bi] = self.pe_n
                        win_last_chunk_pe[(k, g)] = self.pe_n

                        if j == cfg.CPT - 1:
                            # DVE spill for (B, t): frees psum slot
                            self.wait("dve", sem_pe, self.pe_n)
                            if not pass_wait_done:
                                # covers prior-pass part writes / prior-step
                                # epilogue reads (same-engine, monotone)
                                self.wait("dve", sem_dve, self.dve_n)
                                pass_wait_done = True

                            def f(eng, B=B, t=t, slot=slot):
                                pt = part_sb[:, t * D : (t + 1) * D]
                                if B == 0:
                                    nc.vector.tensor_copy(
                                        pt, psum[slot][:, :D]
                                    ).then_inc(sem_dve, 1)
                                else:
                                    nc.vector.tensor_add(
                                        pt, pt, psum[slot][:, :D]
                                    ).then_inc(sem_dve, 1)

                            self.op("dve", f)
                            self.dve_n += 1
                            spill_done[q_global - 1] = self.dve_n

                            if B == cfg.BLOCKS - 1 and (t + 1) % TPQ == 0:
                                emit_quarter_tail(k, t // TPQ)

                blk_guse[(k, B)] = (self.guse[0], self.guse[1])

            # (epilogue + stage + collective handled per quarter above)

        # ---------------- emit ----------------
        prog = self.prog
        with nc.Block() as block:

            @block.sync
            def _(eng):
                for f in prog["sp"]:
                    f(eng)

            @block.gpsimd
            def _(eng):
                for f in prog["gps"]:
                    f(eng)

            @block.tensor
            def _(eng):
                for f in prog["pe"]:
                    f(eng)

            @block.vector
            def _(eng):
                for f in prog["dve"]:
                    f(eng)

        nc.compile()
        ctx.close()
        return nc


def build_graph(cfg: Cfg):
    return Builder(cfg).build()


# ----------------------------------------------------------------------------
# entry point
# ----------------------------------------------------------------------------
def _run_spmd(nc, in_maps, n_cores, iters=1):
    """Compile once via bass2jax custom-call, run `iters` times, return
    (per-core outputs of last run, list of wall times)."""
    import time

    import jax
    from jax.sharding import Mesh, PartitionSpec
    from jax.experimental.shard_map import shard_map

    from concourse import bass2jax, mybir as mb

    bass2jax.install_neuronx_cc_hook()
    partition_name = (
        nc.partition_id_tensor.name if nc.partition_id_tensor else None
    )

    in_names, out_names, out_avals, zero_outs = [], [], [], []
    for alloc in nc.m.functions[0].allocations:
        if not isinstance(alloc, mb.MemoryLocationSet):
            continue
        name = alloc.memorylocations[0].name
        if alloc.kind == "ExternalInput":
            if name != partition_name:
                in_names.append(name)
        elif alloc.kind == "ExternalOutput":
            shape = tuple(alloc.tensor_shape)
            dtype = mb.dt.np(alloc.dtype)
            out_names.append(name)
            out_avals.append(jax.core.ShapedArray(shape, dtype))
            zero_outs.append(np.zeros(shape, dtype))
    n_params = len(in_names)
    in_names = in_names + out_names
    if partition_name is not None:
        in_names.append(partition_name)

    def _body(*args):
        operands = list(args)
        if partition_name is not None:
            operands.append(bass2jax.partition_id_tensor())
        outs = bass2jax._bass_exec_p.bind(
            *operands,
            out_avals=tuple(out_avals),
            in_names=tuple(in_names),
            out_names=tuple(out_names),
            lowering_input_output_aliases=(),
            sim_require_finite=True,
            sim_require_nnan=True,
            nc=nc,
        )
        return tuple(outs)

    devices = jax.devices()[:n_cores]
    mesh = Mesh(np.asarray(devices), ("core",))
    n_outs = len(out_names)
    in_specs = (PartitionSpec("core"),) * (n_params + n_outs)
    out_specs = (PartitionSpec("core"),) * n_outs
    sharded = jax.jit(
        shard_map(
            _body, mesh=mesh, in_specs=in_specs, out_specs=out_specs,
            check_rep=False,
        ),
        keep_unused=True,
    )
    concat_in = [
        np.concatenate([np.asarray(in_maps[c][in_names[i]]) for c in range(n_cores)], axis=0)
        for i in range(n_params)
    ]
    concat_zeros = [
        np.zeros((n_cores * z.shape[0], *z.shape[1:]), z.dtype) for z in zero_outs
    ]
    args = [jax.device_put(a) for a in concat_in + concat_zeros]
    out = jax.block_until_ready(sharded(*args))  # compile + warm
    times = []
    for _ in range(iters):
        t0 = time.perf_counter()
        out = jax.block_until_ready(sharded(*args))
        times.append(time.perf_counter() - t0)
    results = [
        {
            name: np.asarray(out[i]).reshape(n_cores, *out_avals[i].shape)[c]
            for i, name in enumerate(out_names)
        }
        for c in range(n_cores)
    ]
    return results, times


def kernel(x, W, b, src, dst, _profile=False, _iters=1):
    cfg = FULL
    try:
        in_maps, perms = preprocess(x, W, b, src, dst, cfg)
    except RuntimeError:
        # denser graph than expected: fall back to looser group capacity
        from dataclasses import replace

        cfg = replace(cfg, CAP=640, G=6912)
        in_maps, perms = preprocess(x, W, b, src, dst, cfg)
    nc = build_graph(cfg)
    results, times = _run_spmd(nc, in_maps, cfg.C, iters=_iters if _profile else 1)
    full = assemble_output(
        [results[c]["out"] for c in range(cfg.C)], cfg, perms
    )
    kernel.last_exec_time_ns = int(min(times) * 1e9)
    kernel.last_times = times
    return full


# revision 28
# speedup vs baseline: 2.7472x; 2.7472x over previous
"""APPNP (K-step PPR propagation) GNN on 8 Trainium2 NeuronCores.

Strategy (1D node-parallel, per the sharding hint):
  - dst nodes sharded across 8 cores; per core TILES tiles of 128 dst rows.
  - per propagation step each core bulk-gathers source-node feature rows for
    its in-edges with InstDMAGatherAnt (int16 indices -> BLOCKS sub-tables of
    <=32768 rows), reduces the edge messages into per-dst aggregates with
    one-hot selector matmuls on the TensorEngine (selectors built on-device
    via is_equal against an iota tile), applies the APPNP epilogue on the
    VectorEngine, then AllGathers the updated norm-scaled feature shards into
    every core's gather table.

One SPMD Bass graph for all 8 cores; per-core edge index/offset streams are
passed via in_maps.  All edge bookkeeping is host-side numpy.
"""

from dataclasses import dataclass
from contextlib import ExitStack

import numpy as np

import concourse.bass as bass
import concourse.bacc as bacc
import concourse.mybir as mybir
from concourse.ap import AP


# ----------------------------------------------------------------------------
# configuration
# ----------------------------------------------------------------------------
@dataclass(frozen=True)
class Cfg:
    C: int = 8            # cores
    D: int = 64           # feature dim
    PER_RAW: int = 12500  # raw nodes per core
    TILES: int = 108      # dst tiles (of 128) per core
    BLOCKS: int = 4       # src sub-tables (rows must fit int16)
    CAP: int = 512        # slots per (tile, block) group (multiple of 128)
    G: int = 6144         # gather window slots (multiple of 128)
    BPW: int = 7          # chunks per selector-build instruction
    K: int = 10           # propagation steps
    ALPHA: float = 0.1
    # ablation flags (timing experiments only; results become wrong)
    ab_no_coll: bool = False
    ab_no_gather: bool = False
    ab_no_pe: bool = False
    ab_no_build: bool = False

    @property
    def PERP(self):
        return self.TILES * 128

    @property
    def NODES_P(self):
        return self.C * self.PERP

    @property
    def BROWS(self):
        return self.NODES_P // self.BLOCKS

    @property
    def CPT(self):
        return self.CAP // 128

    @property
    def SLOTS_B(self):
        return self.TILES * self.CAP

    @property
    def WIN_B(self):
        assert self.SLOTS_B % self.G == 0
        return self.SLOTS_B // self.G

    @property
    def WSLOT(self):
        return self.G // 128

    @property
    def CW(self):  # matmul chunks per window
        return self.G // 128

    @property
    def NWIN(self):
        return self.BLOCKS * self.WIN_B


FULL = Cfg()
OOB = 500.0  # dstoff sentinel for pad slots (outside iota range 0..127)

F32 = mybir.dt.float32
I16 = mybir.dt.int16


# ----------------------------------------------------------------------------
# host-side preprocessing
# ----------------------------------------------------------------------------
def preprocess(x, W, b, src, dst, cfg: Cfg):
    N, D = x.shape
    assert N == cfg.C * cfg.PER_RAW and D == cfg.D
    src = np.asarray(src, np.int64)
    dst = np.asarray(dst, np.int64)
    x = np.asarray(x, np.float32)
    b = np.asarray(b, np.float32)

    deg_out = np.bincount(src, minlength=N).astype(np.float32)
    deg_in = np.bincount(dst, minlength=N).astype(np.float32)
    ns = np.where(deg_out > 0, 1.0 / np.sqrt(np.maximum(deg_out, 1.0)), 0.0)
    nd = np.where(deg_in > 0, 1.0 / np.sqrt(np.maximum(deg_in, 1.0)), 0.0)
    ns = ns.astype(np.float32)
    nd = nd.astype(np.float32)

    # Balanced dst->tile assignment per core: keep every (tile, block) edge
    # group under CAP so the static SPMD chunk schedule stays tight.
    # node (c, loc) -> (t, p); table row = c*PERP + p*TILES + t.
    dst_c = dst // cfg.PER_RAW
    dst_loc = dst % cfg.PER_RAW
    src_c = src // cfg.PER_RAW
    src_loc = src % cfg.PER_RAW

    # Gather block q == tile-quarter q so each quarter's AllGather can
    # pipeline against the next step's block-q gathers.  A node's quarter is
    # fixed up front (loc // NPQ), so per-dst block-degree vectors are known
    # before tile assignment and balancing stays exact.
    NQ = cfg.BLOCKS
    TPQ = cfg.TILES // NQ
    NPQ = cfg.PER_RAW // NQ
    assert cfg.TILES % NQ == 0 and cfg.PER_RAW % NQ == 0

    perm_t = np.empty((cfg.C, cfg.PER_RAW), np.int64)
    perm_p = np.empty((cfg.C, cfg.PER_RAW), np.int64)
    src_q = src_loc // NPQ  # fixed quarter of each edge's source node
    for c in range(cfg.C):
        sel = np.nonzero(dst_c == c)[0]
        locs = dst_loc[sel]
        degv = np.zeros((cfg.PER_RAW, cfg.BLOCKS), np.int64)
        np.add.at(degv, (locs, src_q[sel]), 1)
        for q in range(NQ):
            lo_n, hi_n = q * NPQ, (q + 1) * NPQ
            order_d = lo_n + np.argsort(-degv[lo_n:hi_n].sum(1), kind="stable")
            loads = np.zeros((TPQ, cfg.BLOCKS), np.int64)
            cnt = np.zeros(TPQ, np.int64)
            for d in order_d:
                v = degv[d]
                scores = (loads + v).max(1)
                scores[cnt >= 128] = 1 << 30
                t = int(np.argmin(scores))
                perm_t[c, d] = q * TPQ + t
                perm_p[c, d] = cnt[t]
                loads[t] += v
                cnt[t] += 1
            if (loads > cfg.CAP).any():
                raise RuntimeError("balance failed")

    # table row (q-major): q*BROWS + c*(128*TPQ) + p*TPQ + (t % TPQ)
    s_t = perm_t[src_c, src_loc]
    s_p = perm_p[src_c, src_loc]
    blk_e = s_t // TPQ
    idx_e = src_c * (128 * TPQ) + s_p * TPQ + (s_t % TPQ)
    assert idx_e.max() < 32768

    dst_t = perm_t[dst_c, dst_loc]
    dst_p = perm_p[dst_c, dst_loc]

    iota = np.tile(np.arange(128, dtype=np.float32), (128, 1))
    brep01 = np.tile(cfg.ALPHA * b[None, :], (128, 1))
    w_in = np.ascontiguousarray(np.asarray(W, np.float32))

    in_maps = []
    for c in range(cfg.C):
        m = dst_c == c
        e_idx = idx_e[m].astype(np.int16)
        e_blk = blk_e[m]
        e_t = dst_t[m]
        e_p = dst_p[m].astype(np.float32)

        key = e_blk * cfg.TILES + e_t
        order = np.argsort(key, kind="stable")
        key_s = key[order]
        counts = np.bincount(key_s, minlength=cfg.BLOCKS * cfg.TILES)
        if counts.max() > cfg.CAP:
            raise RuntimeError(f"group overflow: {counts.max()} > {cfg.CAP}")
        starts = np.concatenate([[0], np.cumsum(counts)[:-1]])
        within = np.arange(len(key_s)) - starts[key_s]
        slot = key_s * cfg.CAP + within

        idx_stream = np.zeros(cfg.BLOCKS * cfg.SLOTS_B, np.int16)
        doff_stream = np.full(cfg.BLOCKS * cfg.SLOTS_B, OOB, np.float32)
        idx_stream[slot] = e_idx[order]
        doff_stream[slot] = e_p[order]

        idx_w = idx_stream.reshape(cfg.NWIN, cfg.G)
        idx16 = idx_w.reshape(cfg.NWIN, cfg.G // 16, 16).transpose(0, 2, 1)
        # Q7 wants the 16-partition index block replicated across all 8
        # gpsimd cores (128 partitions total)
        idx_in = np.ascontiguousarray(np.tile(idx16, (1, 8, 1)))
        doff_w = doff_stream.reshape(cfg.NWIN, cfg.G)
        doff_in = np.ascontiguousarray(
            doff_w.reshape(cfg.NWIN, cfg.WSLOT, 128).transpose(0, 2, 1)
        )

        lo, hi = c * cfg.PER_RAW, (c + 1) * cfg.PER_RAW
        col = perm_t[c] * 128 + perm_p[c]  # node loc -> xT column (t*128+p)
        xT = np.zeros((cfg.D, cfg.PERP), np.float32)
        xT[:, col] = x[lo:hi].T
        ns_in = np.zeros((128, cfg.TILES), np.float32)
        nd9_in = np.zeros((128, cfg.TILES), np.float32)
        ns_in[perm_p[c], perm_t[c]] = ns[lo:hi]
        nd9_in[perm_p[c], perm_t[c]] = (1.0 - cfg.ALPHA) * nd[lo:hi]
        # bns[p, t, c] = b[c] * ns[node(t,p)]  (for fused init epilogue)
        bns = np.ascontiguousarray(
            ns_in[:, :, None] * b[None, None, :]
        )  # [128, TILES, D]

        in_maps.append(
            {
                "xT": xT,
                "w": w_in,
                "brep01": brep01,
                "bns": bns,
                "iota": iota,
                "ns": ns_in,
                "nd9": nd9_in,
                "idx": idx_in,
                "doff": doff_in,
            }
        )
    return in_maps, (perm_t, perm_p)


def assemble_output(outs, cfg: Cfg, perms):
    perm_t, perm_p = perms
    N = cfg.C * cfg.PER_RAW
    full = np.empty((N, cfg.D), np.float32)
    for c in range(cfg.C):
        o = np.asarray(outs[c]).reshape(128, cfg.TILES, cfg.D)
        full[c * cfg.PER_RAW : (c + 1) * cfg.PER_RAW] = o[perm_p[c], perm_t[c]]
    return full


# ----------------------------------------------------------------------------
# graph builder
# ----------------------------------------------------------------------------
class Builder:
    def __init__(self, cfg: Cfg):
        self.cfg = cfg
        self.nc = bacc.Bacc("TRN2", target_bir_lowering=False)
        self.prog = {"sp": [], "gps": [], "pe": [], "dve": []}
        self.sp_n = 0
        self.pe_n = 0
        self.dve_n = 0
        self.cc_n = 0
        self.guse = [0, 0]
        self.uses_idx = [0, 0]
        self.uses_doff = [0, 0]
        self.st_n = 0
        self._last_wait = {}

    def op(self, engine, fn):
        self.prog[engine].append(fn)

    def wait(self, engine, s, v):
        """Emit wait_ge, deduping against earlier >= waits on this engine."""
        if v <= 0:
            return
        key = (engine, id(s))
        if self._last_wait.get(key, -1) >= v:
            return
        self._last_wait[key] = v
        self.prog[engine].append(lambda eng, s=s, v=v: eng.wait_ge(s, v))

    def build(self):
        cfg, nc = self.cfg, self.nc
        D = cfg.D

        xT_d = nc.declare_dram_parameter("xT", [D, cfg.PERP], F32, isOutput=False)
        w_d = nc.declare_dram_parameter("w", [D, D], F32, isOutput=False)
        brep01_d = nc.declare_dram_parameter("brep01", [128, D], F32, isOutput=False)
        bns_d = nc.declare_dram_parameter(
            "bns", [128, cfg.TILES, D], F32, isOutput=False
        )
        iota_d = nc.declare_dram_parameter("iota", [128, 128], F32, isOutput=False)
        ns_d = nc.declare_dram_parameter("ns", [128, cfg.TILES], F32, isOutput=False)
        nd9_d = nc.declare_dram_parameter("nd9", [128, cfg.TILES], F32, isOutput=False)
        idx_d = nc.declare_dram_parameter(
            "idx", [cfg.NWIN, 128, cfg.G // 16], I16, isOutput=False
        )
        doff_d = nc.declare_dram_parameter(
            "doff", [cfg.NWIN, 128, cfg.WSLOT], F32, isOutput=False
        )
        out_d = nc.declare_dram_parameter("out", [128, cfg.TILES, D], F32, isOutput=True)

        TPQ0 = cfg.TILES // cfg.BLOCKS
        shard_q = [
            nc.dram_tensor(f"shard{q}", [128, TPQ0, D], F32)
            for q in range(cfg.BLOCKS)
        ]
        table_d = nc.dram_tensor("table", [cfg.NODES_P, D], F32, addr_space="Shared")

        ctx = ExitStack()
        sb = lambda name, shape, dt=F32: ctx.enter_context(
            nc.sbuf_tensor(name, shape, dt)
        )
        w_sb = sb("wsb", [64, D])
        brep01_sb = sb("brep01sb", [128, D])
        bns_sb = sb("bnssb", [128, cfg.TILES * D])
        iota_sb = sb("iotasb", [128, 128])
        ns_sb = sb("nssb", [128, cfg.TILES])
        nd9_sb = sb("nd9sb", [128, cfg.TILES])
        h0a_sb = sb("h0a", [128, cfg.TILES * D])
        part_sb = sb("part", [128, cfg.TILES * D])
        stage_sb = sb("stage", [128, cfg.TILES * D])
        NPS = 8
        NS8 = 4
        NMSG = 4
        msg_sb = [sb(f"msg{i}", [128, cfg.WSLOT, D]) for i in range(2)]
        s8_sb = [sb(f"s8_{i}", [128, cfg.BPW * 128]) for i in range(NS8)]
        doff_sb = [sb(f"doffsb{i}", [128, cfg.WSLOT]) for i in range(2)]
        idx_sb = [sb(f"idxsb{i}", [128, cfg.G // 16], I16) for i in range(2)]
        # xT is init-only: allocate it on top of the SBUF stack, free it after
        # the other tiles, and reuse its range for the extra message buffers
        # (the init->gather sem chain orders the accesses).
        ctx_xt = ExitStack()
        xt_sb = ctx_xt.enter_context(nc.sbuf_tensor("xt", [64, cfg.PERP], F32))
        ctx_xt.close()
        msg_sb += [sb(f"msg{i}", [128, cfg.WSLOT, D]) for i in range(2, NMSG)]
        psum = [
            ctx.enter_context(nc.psum_tensor(f"ps{i}", [128, 512], F32))
            for i in range(NPS)
        ]

        sem_sp = ctx.enter_context(nc.semaphore("s_sp"))
        sem_g = [
            ctx.enter_context(nc.semaphore("s_g0")),
            ctx.enter_context(nc.semaphore("s_g1")),
        ]
        sem_idx = [
            ctx.enter_context(nc.semaphore("s_ix0")),
            ctx.enter_context(nc.semaphore("s_ix1")),
        ]
        sem_doff = [
            ctx.enter_context(nc.semaphore("s_do0")),
            ctx.enter_context(nc.semaphore("s_do1")),
        ]
        sem_st = [
            ctx.enter_context(nc.semaphore(f"s_st{i}")) for i in range(cfg.BLOCKS)
        ]
        sem_cc = ctx.enter_context(nc.semaphore("s_cc"))
        sem_pe = ctx.enter_context(nc.semaphore("s_pe"))
        sem_dve = ctx.enter_context(nc.semaphore("s_dve"))

        # gpsimd: load the Q7 library that provides DMAGatherAnt; pin the
        # num_idxs register once (to_reg per gather would exhaust the pool)
        holder = {}

        def f(eng):
            from concourse import library_config

            nc.gpsimd.load_library(library_config.mlp)
            holder["greg"] = nc.gpsimd.to_reg(cfg.G)

        self.op("gps", f)

        # ---------------- init: constant DMAs ----------------
        def sp_dma(mk):
            def f(eng, mk=mk):
                out_ap, in_ap = mk()
                nc.sync.dma_start(out=out_ap, in_=in_ap).then_inc(sem_sp, 16)

            self.op("sp", f)
            self.sp_n += 16
            return self.sp_n

        sp_dma(lambda: (xt_sb[:, :], xT_d[:, :]))
        sp_dma(lambda: (w_sb[:, :], w_d[:, :]))
        sp_dma(lambda: (brep01_sb[:, :], brep01_d[:, :]))
        sp_dma(
            lambda: (
                bns_sb[:, :].rearrange("p (t c) -> p t c", c=D),
                bns_d[:, :, :],
            )
        )
        sp_dma(lambda: (iota_sb[:, :], iota_d[:, :]))
        sp_dma(lambda: (ns_sb[:, :], ns_d[:, :]))
        init_dma_done = sp_dma(lambda: (nd9_sb[:, :], nd9_d[:, :]))

        # ---------------- init: h0 = x @ W + b; seed stage/h0a ----------------
        init_tile_done = {}
        self.wait("pe", sem_sp, init_dma_done)
        for t in range(cfg.TILES):
            slot = t % NPS
            if t >= NPS:
                self.wait("pe", sem_dve, init_tile_done[t - NPS])

            def f(eng, t=t, slot=slot):
                nc.tensor.matmul(
                    out=psum[slot][:, :D],
                    lhsT=xt_sb[:, t * 128 : (t + 1) * 128],
                    rhs=w_sb[:, :],
                    start=True,
                    stop=True,
                ).then_inc(sem_pe, 1)

            self.op("pe", f)
            self.pe_n += 1

            self.wait("dve", sem_pe, self.pe_n)
            self.wait("dve", sem_sp, init_dma_done)

            def f(eng, t=t, slot=slot):
                # h0a_t = alpha*(xW) + alpha*b ; stage_t = ns*(xW) + ns*b
                nc.vector.scalar_tensor_tensor(
                    out=h0a_sb[:, t * D : (t + 1) * D],
                    in0=psum[slot][:, :D],
                    scalar=cfg.ALPHA,
                    in1=brep01_sb[:, :],
                    op0=mybir.AluOpType.mult,
                    op1=mybir.AluOpType.add,
                ).then_inc(sem_dve, 1)
                nc.vector.scalar_tensor_tensor(
                    out=stage_sb[:, t * D : (t + 1) * D],
                    in0=psum[slot][:, :D],
                    scalar=ns_sb[:, t : t + 1],
                    in1=bns_sb[:, t * D : (t + 1) * D],
                    op0=mybir.AluOpType.mult,
                    op1=mybir.AluOpType.add,
                ).then_inc(sem_dve, 1)

            self.op("dve", f)
            self.dve_n += 2
            init_tile_done[t] = self.dve_n
        init_done_dve = self.dve_n

        TPQ = cfg.TILES // cfg.BLOCKS
        self.st_q = [0] * cfg.BLOCKS

        def stage_quarter(dram3, q):
            # dram3 None -> this quarter's shard tensor; else out_d slice
            a, b2 = q * TPQ, (q + 1) * TPQ

            def f(eng, dram3=dram3, q=q, a=a, b2=b2):
                out_ap = (
                    shard_q[q][:, :, :] if dram3 is None else dram3[:, a:b2, :]
                )
                nc.sync.dma_start(
                    out=out_ap,
                    in_=stage_sb[:, a * D : b2 * D].rearrange(
                        "p (t c) -> p t c", c=D
                    ),
                ).then_inc(sem_st[q], 16)

            self.op("sp", f)
            self.st_q[q] += 16
            return self.st_q[q]

        for q in range(cfg.BLOCKS):
            self.wait("sp", sem_dve, init_tile_done[(q + 1) * TPQ - 1])
            stage_quarter(None, q)

        # ---------------- propagation steps ----------------
        win_last_chunk_pe = {}
        build_done = {}
        build_last_pe = {}
        build_of_chunk = {}
        last_doff_builds = {}
        spill_done = {}
        bi_global = 0
        q_global = 0

        blk_guse = {}

        def emit_quarter_tail(k, qq):
            # epilogue waves + stage dma for tile quarter qq of step k;
            # runs while the B3 pass continues on later quarters.
            a = qq * TPQ
            self.wait("dve", sem_dve, self.dve_n)     # B3 adds retired
            self.wait("dve", sem_st[qq], 16 * k)      # stage quarter free
            for t in range(a, a + TPQ):

                def f(eng, t=t):
                    nc.vector.scalar_tensor_tensor(
                        out=stage_sb[:, t * D : (t + 1) * D],
                        in0=part_sb[:, t * D : (t + 1) * D],
                        scalar=nd9_sb[:, t : t + 1],
                        in1=h0a_sb[:, t * D : (t + 1) * D],
                        op0=mybir.AluOpType.mult,
                        op1=mybir.AluOpType.add,
                    ).then_inc(sem_dve, 1)

                self.op("dve", f)
                self.dve_n += 1
            if k < cfg.K:
                self.wait("dve", sem_dve, self.dve_n)  # wave barrier
                for t in range(a, a + TPQ):

                    def f(eng, t=t):
                        nc.vector.tensor_scalar_mul(
                            stage_sb[:, t * D : (t + 1) * D],
                            stage_sb[:, t * D : (t + 1) * D],
                            ns_sb[:, t : t + 1],
                        ).then_inc(sem_dve, 1)

                    self.op("dve", f)
                    self.dve_n += 1
            # SP: stage quarter (shard for next step, out on last step)
            self.wait("sp", sem_dve, self.dve_n)
            if not cfg.ab_no_coll:
                self.wait("sp", sem_cc, 4 * (k - 1) + qq + 1)  # shard-q WAR
            stage_quarter(None if k < cfg.K else out_d, qq)

        for k in range(1, cfg.K + 1):
            for B in range(cfg.BLOCKS):
                # quarter collective B: shard quarter -> table block rows.
                # Pipelines: issued just before this block's gathers; waits
                # only on quarter-B stage of step k-1 and on step k-1's
                # block-B gathers having drained (table WAR).
                self.wait("gps", sem_st[B], 16 * k)
                snap = blk_guse.get((k - 1, B))
                if snap is not None:
                    self.wait("gps", sem_g[0], 16 * snap[0])
                    self.wait("gps", sem_g[1], 16 * snap[1])

                def f(eng, B=B):
                    if cfg.ab_no_coll:
                        return
                    nc.gpsimd.collective_compute(
                        "AllGather",
                        mybir.AluOpType.bypass,
                        replica_groups=[list(range(cfg.C))],
                        ins=[shard_q[B].ap().opt()],
                        outs=[
                            table_d[
                                B * cfg.BROWS : (B + 1) * cfg.BROWS, :
                            ].opt()
                        ],
                    ).then_inc(sem_cc, 1)

                self.op("gps", f)
                if not cfg.ab_no_coll:
                    self.cc_n += 1
                cc_now = self.cc_n

                pass_wait_done = False
                for w in range(cfg.WIN_B):
                    g = B * cfg.WIN_B + w
                    par = g % 2
                    buf = g % NMSG
                    prev = (k, g - 2) if g - 2 >= 0 else (k - 1, g - 2 + cfg.NWIN)
                    prevb = (
                        (k, g - NMSG)
                        if g - NMSG >= 0
                        else (k - 1, g - NMSG + cfg.NWIN)
                    )

                    # SP: idx window dma (WAR: last gather on this parity done)
                    self.wait("sp", sem_g[par], 16 * self.guse[par])

                    def f(eng, g=g, par=par):
                        nc.sync.dma_start(
                            out=idx_sb[par][:, :], in_=idx_d[g, :, :]
                        ).then_inc(sem_idx[par], 16)

                    self.op("sp", f)
                    self.uses_idx[par] += 16
                    idx_dma_done = self.uses_idx[par]

                    # SP: doff window dma (WAR: builds of window g-2 done)
                    if prev in last_doff_builds:
                        self.wait("sp", sem_dve, build_done[last_doff_builds[prev]])

                    def f(eng, g=g, par=par):
                        nc.sync.dma_start(
                            out=doff_sb[par][:, :], in_=doff_d[g, :, :]
                        ).then_inc(sem_doff[par], 16)

                    self.op("sp", f)
                    self.uses_doff[par] += 16
                    doff_dma_done = self.uses_doff[par]

                    # GPS: gather (in-flight cap 2 via parity sem; buffer
                    # WAR vs window g-NMSG)
                    self.wait("gps", sem_cc, cc_now)
                    self.wait("gps", sem_idx[par], idx_dma_done)
                    self.wait("gps", sem_g[par], 16 * self.guse[par])
                    if prevb in win_last_chunk_pe:
                        self.wait("gps", sem_pe, win_last_chunk_pe[prevb])

                    def f(eng, B=B, par=par, buf=buf):
                        if cfg.ab_no_gather:
                            nc.gpsimd.memset(msg_sb[buf][0:1, 0:1, 0:1], 0).then_inc(
                                sem_g[par], 16
                            )
                            return
                        nc.gpsimd.dma_gather(
                            out_ap=msg_sb[buf][:, :, :],
                            in_ap=table_d[B * cfg.BROWS : (B + 1) * cfg.BROWS, :],
                            idxs_ap=idx_sb[par][:, :],
                            num_idxs=cfg.G,
                            num_idxs_reg=holder["greg"],
                            elem_size=D,
                            single_packet=False,
                        ).then_inc(sem_g[par], 16)

                    self.op("gps", f)
                    self.guse[par] += 1
                    g_thresh = 16 * self.guse[par]

                    # DVE builds + PE chunks + DVE spills, interleaved
                    first_c = w * cfg.CW
                    for cw in range(cfg.CW):
                        if cw % cfg.BPW == 0:
                            bw = cw // cfg.BPW
                            nch = min(cfg.BPW, cfg.CW - cw)
                            sbi = bi_global % NS8
                            self.wait("dve", sem_doff[par], doff_dma_done)
                            if (bi_global - NS8) in build_last_pe:
                                self.wait(
                                    "dve", sem_pe, build_last_pe[bi_global - NS8]
                                )

                            def f(eng, par=par, bw=bw, nch=nch, sbi=sbi):
                                if cfg.ab_no_build:
                                    nc.vector.memset(
                                        s8_sb[sbi][0:1, 0:1], 0
                                    ).then_inc(sem_dve, 1)
                                    return
                                s8 = s8_sb[sbi][:, :]
                                out_ap = AP(
                                    tensor=s8.tensor,
                                    offset=s8.offset,
                                    ap=[list(s8.ap[0]), [128, nch], [1, 128]],
                                )
                                in0 = doff_sb[par][
                                    :, bw * cfg.BPW : bw * cfg.BPW + nch
                                ].to_broadcast([128, nch, 128])
                                io = iota_sb[:, :]
                                in1 = AP(
                                    tensor=io.tensor,
                                    offset=io.offset,
                                    ap=[list(io.ap[0]), [0, nch], [1, 128]],
                                )
                                nc.vector.tensor_tensor(
                                    out=out_ap,
                                    in0=in0,
                                    in1=in1,
                                    op=mybir.AluOpType.is_equal,
                                ).then_inc(sem_dve, 1)

                            self.op("dve", f)
                            self.dve_n += 1
                            build_done[bi_global] = self.dve_n
                            for cc2 in range(cw, cw + nch):
                                build_of_chunk[(k, g, cc2)] = bi_global
                            last_doff_builds[(k, g)] = bi_global
                            bi_global += 1

                        c_glob = first_c + cw
                        t = c_glob // cfg.CPT
                        j = c_glob % cfg.CPT
                        bi = build_of_chunk[(k, g, cw)]
                        sbi = bi % NS8
                        self.wait("pe", sem_g[par], g_thresh)
                        self.wait("pe", sem_dve, build_done[bi])
                        if j == 0:
                            qh = q_global - NPS
                            if qh >= 0:
                                self.wait("pe", sem_dve, spill_done[qh])
                            else:
                                self.wait("pe", sem_dve, init_done_dve)
                            q_global += 1
                        slot = (q_global - 1) % NPS

                        def f(eng, buf=buf, cw=cw, j=j, sbi=sbi, slot=slot):
                            if cfg.ab_no_pe:
                                nc.tensor.memset(psum[slot][0:1, 0:1], 0).then_inc(
                                    sem_pe, 1
                                )
                                return
                            col = (cw % cfg.BPW) * 128
                            nc.tensor.matmul(
                                out=psum[slot][:, :D],
                                lhsT=s8_sb[sbi][:, col : col + 128],
                                rhs=msg_sb[buf][:, cw, :],
                                start=(j == 0),
                                stop=(j == cfg.CPT - 1),
                            ).then_inc(sem_pe, 1)

                        self.op("pe", f)
                        self.pe_n += 1
                        build_last_pe[bi] = self.pe_n
                        win_last_chunk_pe[(k, g)] = self.pe_n

                        if j == cfg.CPT - 1:
                            # DVE spill for (B, t): frees psum slot
                            self.wait("dve", sem_pe, self.pe_n)
                            if not pass_wait_done:
                                # covers prior-pass part writes / prior-step
                                # epilogue reads (same-engine, monotone)
                                self.wait("dve", sem_dve, self.dve_n)
                                pass_wait_done = True

                            def f(eng, B=B, t=t, slot=slot):
                                pt = part_sb[:, t * D : (t + 1) * D]
                                if B == 0:
                                    nc.vector.tensor_copy(
                                        pt, psum[slot][:, :D]
                                    ).then_inc(sem_dve, 1)
                                else:
                                    nc.vector.tensor_add(
                                        pt, pt, psum[slot][:, :D]
                                    ).then_inc(sem_dve, 1)

                            self.op("dve", f)
                            self.dve_n += 1
                            spill_done[q_global - 1] = self.dve_n

                            if B == cfg.BLOCKS - 1 and (t + 1) % TPQ == 0:
                                emit_quarter_tail(k, t // TPQ)

                blk_guse[(k, B)] = (self.guse[0], self.guse[1])

            # (epilogue + stage + collective handled per quarter above)

        # ---------------- emit ----------------
        prog = self.prog
        with nc.Block() as block:

            @block.sync
            def _(eng):
                for f in prog["sp"]:
                    f(eng)

            @block.gpsimd
            def _(eng):
                for f in prog["gps"]:
                    f(eng)

            @block.tensor
            def _(eng):
                for f in prog["pe"]:
                    f(eng)

            @block.vector
            def _(eng):
                for f in prog["dve"]:
                    f(eng)

        nc.compile()
        ctx.close()
        return nc


def build_graph(cfg: Cfg):
    return Builder(cfg).build()


# ----------------------------------------------------------------------------
# entry point
# ----------------------------------------------------------------------------
def _run_spmd(nc, in_maps, n_cores, iters=1):
    """Compile once via bass2jax custom-call, run `iters` times, return
    (per-core outputs of last run, list of wall times)."""
    import time

    import jax
    from jax.sharding import Mesh, PartitionSpec
    from jax.experimental.shard_map import shard_map

    from concourse import bass2jax, mybir as mb

    bass2jax.install_neuronx_cc_hook()
    partition_name = (
        nc.partition_id_tensor.name if nc.partition_id_tensor else None
    )

    in_names, out_names, out_avals, zero_outs = [], [], [], []
    for alloc in nc.m.functions[0].allocations:
        if not isinstance(alloc, mb.MemoryLocationSet):
            continue
        name = alloc.memorylocations[0].name
        if alloc.kind == "ExternalInput":
            if name != partition_name:
                in_names.append(name)
        elif alloc.kind == "ExternalOutput":
            shape = tuple(alloc.tensor_shape)
            dtype = mb.dt.np(alloc.dtype)
            out_names.append(name)
            out_avals.append(jax.core.ShapedArray(shape, dtype))
            zero_outs.append(np.zeros(shape, dtype))
    n_params = len(in_names)
    in_names = in_names + out_names
    if partition_name is not None:
        in_names.append(partition_name)

    def _body(*args):
        operands = list(args)
        if partition_name is not None:
            operands.append(bass2jax.partition_id_tensor())
        outs = bass2jax._bass_exec_p.bind(
            *operands,
            out_avals=tuple(out_avals),
            in_names=tuple(in_names),
            out_names=tuple(out_names),
            lowering_input_output_aliases=(),
            sim_require_finite=True,
            sim_require_nnan=True,
            nc=nc,
        )
        return tuple(outs)

    devices = jax.devices()[:n_cores]
    mesh = Mesh(np.asarray(devices), ("core",))
    n_outs = len(out_names)
    in_specs = (PartitionSpec("core"),) * (n_params + n_outs)
    out_specs = (PartitionSpec("core"),) * n_outs
    sharded = jax.jit(
        shard_map(
            _body, mesh=mesh, in_specs=in_specs, out_specs=out_specs,
            check_rep=False,
        ),
        keep_unused=True,
    )
    concat_in = [
        np.concatenate([np.asarray(in_maps[c][in_names[i]]) for c in range(n_cores)], axis=0)
        for i in range(n_params)
    ]
    concat_zeros = [
        np.zeros((n_cores * z.shape[0], *z.shape[1:]), z.dtype) for z in zero_outs
    ]
    args = [jax.device_put(a) for a in concat_in + concat_zeros]
    out = jax.block_until_ready(sharded(*args))  # compile + warm
    times = []
    for _ in range(iters):
        t0 = time.perf_counter()
        out = jax.block_until_ready(sharded(*args))
        times.append(time.perf_counter() - t0)
    results = [
        {
            name: np.asarray(out[i]).reshape(n_cores, *out_avals[i].shape)[c]
            for i, name in enumerate(out_names)
        }
        for c in range(n_cores)
    ]
    return results, times


def kernel(x, W, b, src, dst, _profile=False, _iters=1):
    cfg = FULL
    try:
        in_maps, perms = preprocess(x, W, b, src, dst, cfg)
    except RuntimeError:
        # denser graph than expected: fall back to looser group capacity
        from dataclasses import replace

        cfg = replace(cfg, CAP=640, G=6912)
        in_maps, perms = preprocess(x, W, b, src, dst, cfg)
    nc = build_graph(cfg)
    results, times = _run_spmd(nc, in_maps, cfg.C, iters=_iters if _profile else 1)
    full = assemble_output(
        [results[c]["out"] for c in range(cfg.C)], cfg, perms
    )
    kernel.last_exec_time_ns = int(min(times) * 1e9)
    kernel.last_times = times
    return full
